# revision 6
# baseline (speedup 1.0000x reference)
"""Cumulative VWAP kernel for Trainium2 (Bass/Tile), data-parallel over 8 cores.

vwap[:, t] = cumsum(s*v)[:, t] / (cumsum(v)[:, t] + 1e-8),  vwap[:, 0] = s[:, 0]

Sharding: num_paths (axis 0) split evenly across 8 NeuronCores; the cumsum
runs along the time axis, which stays local to each core (no collectives).

Key engine facts driving the design (all HW-measured on trn2):
  * The builtin DVE tensor_tensor_scan runs at ~2 cycles/element (two ALU ops
    in the recurrence).  A custom DVE op (dve_spec `scan()` node) runs at
    ~1 element/cycle and can fuse the s*v multiply into the scan input
    (feed-forward stage), so pv_cum costs one op and no separate multiply.
  * GpSimd cannot run scans (ISA rejects opcode 0xe5 on Pool) and serializes
    against DVE scans on the shared SBUF port pair -> GpSimd unused.
  * ACT Reciprocal is banned (accuracy); 1/x = exp(-ln(x)) on ACT instead,
    with both functions forced into ONE activation-table set (otherwise the
    table-load pass alternates natural_log/exp_and_others every tile,
    ~2.7us per reload).
  * eps=1e-8 is a fp32 no-op since v_cum >= 1e6 (ulp >= 1/16).

Per-core dataflow, per [128, 4096] tile (16 tiles per core):
  DMA  : load stock tile, load volume tile          (2 MiB each, contiguous)
  ACT  : save col 0 of stock (the t==0 fix is an exact copy of s0)
  DVE  : pv_cum = custom scan(ADD, s*v)             (~4.3 us, in place)
  DVE  : v_cum  = custom scan(ADD, v)               (~4.2 us, in place)
  ACT  : r = exp(-ln(v_cum))                        (2 ops, one table set)
  DVE  : vwap = pv_cum * r                          (tensor_tensor, ~4.4 us)
  ACT  : restore col 0
  DMA  : store vwap tile
Engine busy per core: DVE ~220us, ACT ~130us, DMA ~270us -> DMA-bound.
"""

import numpy as np

NUM_PATHS = 16384
TIME = 4096
N_CORES = 8
ROWS = NUM_PATHS // N_CORES  # rows per core
P = 128  # SBUF partitions

_CACHE = {}

_COMBINED_SET = "natural_log_exp_and_others"


def _single_act_set_bacc():
    import concourse.bacc as bacc

    class SingleActSetBacc(bacc.Bacc):
        """Restrict the activation-table-load pass to one set holding
        Ln+Exp+Copy so alternating Ln/Exp doesn't reload tables every tile."""

        def insert_act_table_loads(self):
            import bass_rust
            import concourse.mybir as mybir
            from concourse.hw_specs import get_activation_tables

            has_activation = any(
                isinstance(i, mybir.InstActivation)
                for b in self.main_func.blocks
                for i in b.instructions
            )
            if not has_activation:
                return
            tables = [
                (name, fns if name == _COMBINED_SET else set())
                for name, fns in get_activation_tables(self.m.arch).items()
            ]
            bass_rust.insert_act_table_loads(self, tables)

    return SingleActSetBacc


def _register_custom_ops():
    """Register the two custom DVE cumsum ops (idempotent)."""
    import concourse.dve_ops as dve_ops
    from concourse.dve_ops import DveOp
    from concourse.dve_spec import AluOp, Spec, Src0, Src1, lower, scan, spec_leaves
    from concourse.dve_uop import DveOpSpec

    def register(name, spec):
        for o in dve_ops.OPS:
            if o.name == name:
                return o
        op = DveOp(name, spec, subdim=False, uops_sha={})
        dve_ops.OPS.append(op)
        dve_ops.CUSTOM_DVE_SPECS[name] = spec
        dve_ops._SUB_OPCODE_FOR_NAME[name] = (
            dve_ops._CUSTOM_DVE_ROW_BASE + len(dve_ops.OPS) - 1
        )
        assert dve_ops._SUB_OPCODE_FOR_NAME[name] < 0x20
        # self-pin the uop hashes (same computation DveOp.compile checks)
        for ver in ("v3", "v4"):
            s = DveOpSpec(
                name=name,
                opcode=dve_ops.get_dve_sub_opcode(name),
                uops=lower(spec, ver=ver),
                rd1_en=Src1 in spec_leaves(spec),
            )
            op.uops_sha[ver] = s.sha(ver)
        return op

    pv = register(
        "PV_CUMSUM_ANT",
        Spec(
            body=scan(AluOp.ADD, Src0 * Src1),
            reference=lambda in0, in1, s0, s1, imm2: np.cumsum(
                in0.astype(np.float32) * in1.astype(np.float32),
                axis=-1, dtype=np.float32,
            ),
        ),
    )
    v = register(
        "V_CUMSUM_ANT",
        Spec(
            body=scan(AluOp.ADD, Src0),
            reference=lambda in0, in1, s0, s1, imm2: np.cumsum(
                in0, axis=-1, dtype=np.float32
            ),
        ),
    )
    return pv, v


def _build(rows=ROWS, time=TIME, bufs=4, reps=1):
    import concourse.tile as tile
    import concourse.mybir as mybir

    pv_op, v_op = _register_custom_ops()
    nc = _single_act_set_bacc()("TRN2", target_bir_lowering=False, debug=False)
    f32 = mybir.dt.float32
    stock = nc.dram_tensor("stock_paths", [rows, time], f32, kind="ExternalInput").ap()
    vol = nc.dram_tensor("volume_paths", [rows, time], f32, kind="ExternalInput").ap()
    out = nc.dram_tensor("vwap_out", [rows, time], f32, kind="ExternalOutput").ap()

    Ln = mybir.ActivationFunctionType.Ln
    Exp = mybir.ActivationFunctionType.Exp

    n_tiles = rows // P
    with tile.TileContext(nc) as tc:
        with (
            tc.tile_pool(name="big", bufs=bufs) as big,
            tc.tile_pool(name="small", bufs=bufs) as small,
        ):
            for i in range(n_tiles * reps):
                r0 = (i % n_tiles) * P
                ts = big.tile([P, time], f32, tag="ts")
                tv = big.tile([P, time], f32, tag="tv")
                nc.sync.dma_start(ts[:], stock[r0 : r0 + P, :])
                nc.sync.dma_start(tv[:], vol[r0 : r0 + P, :])
                t0 = small.tile([P, 1], f32, tag="t0")
                nc.scalar.copy(t0[:], ts[:, 0:1])
                nc.vector._custom_dve(pv_op, out=ts[:], in0=ts[:], in1=tv[:])
                nc.vector._custom_dve(v_op, out=tv[:], in0=tv[:])
                nc.scalar.activation(tv[:], tv[:], Ln)
                nc.scalar.activation(tv[:], tv[:], Exp, scale=-1.0)
                nc.vector.tensor_mul(ts[:], ts[:], tv[:])  # vwap
                nc.scalar.copy(ts[:, 0:1], t0[:])
                nc.sync.dma_start(out[r0 : r0 + P, :], ts[:])
    nc.compile()
    return nc


def _get_nc():
    if "nc" not in _CACHE:
        _CACHE["nc"] = _build()
    return _CACHE["nc"]


def kernel(stock_paths: np.ndarray, volume_paths: np.ndarray) -> np.ndarray:
    from concourse.bass_utils import run_bass_kernel_spmd

    stock_paths = np.ascontiguousarray(stock_paths, dtype=np.float32)
    volume_paths = np.ascontiguousarray(volume_paths, dtype=np.float32)
    assert stock_paths.shape == (NUM_PATHS, TIME)

    nc = _get_nc()
    in_maps = [
        {
            "stock_paths": stock_paths[i * ROWS : (i + 1) * ROWS],
            "volume_paths": volume_paths[i * ROWS : (i + 1) * ROWS],
        }
        for i in range(N_CORES)
    ]
    res = run_bass_kernel_spmd(nc, in_maps, core_ids=list(range(N_CORES)))
    return np.concatenate([r["vwap_out"] for r in res.results], axis=0)


# revision 7
# speedup vs baseline: 1.0268x; 1.0268x over previous
"""Cumulative VWAP kernel for Trainium2 (Bass/Tile), data-parallel over 8 cores.

vwap[:, t] = cumsum(s*v)[:, t] / (cumsum(v)[:, t] + 1e-8),  vwap[:, 0] = s[:, 0]

Sharding: num_paths (axis 0) split evenly across 8 NeuronCores; the cumsum
runs along the time axis, which stays local to each core (no collectives).

Key engine facts driving the design (all HW-measured on trn2):
  * The builtin DVE tensor_tensor_scan runs at ~2 cycles/element (two ALU ops
    in the recurrence).  A custom DVE op (dve_spec `scan()` node) runs at
    ~1 element/cycle and can fuse the s*v multiply into the scan input
    (feed-forward stage), so pv_cum costs one op and no separate multiply.
  * GpSimd cannot run scans (ISA rejects opcode 0xe5 on Pool) and serializes
    against DVE scans on the shared SBUF port pair -> GpSimd unused.
  * ACT Reciprocal is banned (accuracy); 1/x = exp(-ln(x)) on ACT instead,
    with both functions forced into ONE activation-table set (otherwise the
    table-load pass alternates natural_log/exp_and_others every tile,
    ~2.7us per reload).
  * eps=1e-8 is a fp32 no-op since v_cum >= 1e6 (ulp >= 1/16).

Per-core dataflow, per [128, 4096] tile (16 tiles per core):
  DMA  : load stock tile, load volume tile          (2 MiB each, contiguous)
  ACT  : save col 0 of stock (the t==0 fix is an exact copy of s0)
  DVE  : pv_cum = custom scan(ADD, s*v)             (~4.3 us, in place)
  DVE  : v_cum  = custom scan(ADD, v)               (~4.2 us, in place)
  ACT  : r = exp(-ln(v_cum))                        (2 ops, one table set)
  DVE  : vwap = pv_cum * r                          (tensor_tensor, ~4.4 us)
  ACT  : restore col 0
  DMA  : store vwap tile
Engine busy per core: DVE ~220us, ACT ~130us, DMA ~270us -> DMA-bound.
"""

import numpy as np

NUM_PATHS = 16384
TIME = 4096
N_CORES = 8
ROWS = NUM_PATHS // N_CORES  # rows per core
P = 128  # SBUF partitions

_CACHE = {}

_COMBINED_SET = "natural_log_exp_and_others"


def _single_act_set_bacc():
    import concourse.bacc as bacc

    class SingleActSetBacc(bacc.Bacc):
        """Restrict the activation-table-load pass to one set holding
        Ln+Exp+Copy so alternating Ln/Exp doesn't reload tables every tile."""

        def insert_act_table_loads(self):
            import bass_rust
            import concourse.mybir as mybir
            from concourse.hw_specs import get_activation_tables

            has_activation = any(
                isinstance(i, mybir.InstActivation)
                for b in self.main_func.blocks
                for i in b.instructions
            )
            if not has_activation:
                return
            tables = [
                (name, fns if name == _COMBINED_SET else set())
                for name, fns in get_activation_tables(self.m.arch).items()
            ]
            bass_rust.insert_act_table_loads(self, tables)

    return SingleActSetBacc


def _register_custom_ops():
    """Register the two custom DVE cumsum ops (idempotent)."""
    import concourse.dve_ops as dve_ops
    from concourse.dve_ops import DveOp
    from concourse.dve_spec import AluOp, Spec, Src0, Src1, lower, scan, spec_leaves
    from concourse.dve_uop import DveOpSpec

    def register(name, spec):
        for o in dve_ops.OPS:
            if o.name == name:
                return o
        op = DveOp(name, spec, subdim=False, uops_sha={})
        dve_ops.OPS.append(op)
        dve_ops.CUSTOM_DVE_SPECS[name] = spec
        dve_ops._SUB_OPCODE_FOR_NAME[name] = (
            dve_ops._CUSTOM_DVE_ROW_BASE + len(dve_ops.OPS) - 1
        )
        assert dve_ops._SUB_OPCODE_FOR_NAME[name] < 0x20
        # self-pin the uop hashes (same computation DveOp.compile checks)
        for ver in ("v3", "v4"):
            s = DveOpSpec(
                name=name,
                opcode=dve_ops.get_dve_sub_opcode(name),
                uops=lower(spec, ver=ver),
                rd1_en=Src1 in spec_leaves(spec),
            )
            op.uops_sha[ver] = s.sha(ver)
        return op

    pv = register(
        "PV_CUMSUM_ANT",
        Spec(
            body=scan(AluOp.ADD, Src0 * Src1),
            reference=lambda in0, in1, s0, s1, imm2: np.cumsum(
                in0.astype(np.float32) * in1.astype(np.float32),
                axis=-1, dtype=np.float32,
            ),
        ),
    )
    v = register(
        "V_CUMSUM_ANT",
        Spec(
            body=scan(AluOp.ADD, Src0),
            reference=lambda in0, in1, s0, s1, imm2: np.cumsum(
                in0, axis=-1, dtype=np.float32
            ),
        ),
    )
    return pv, v


def _build(rows=ROWS, time=TIME, bufs=4, reps=1, width=None):
    import concourse.tile as tile
    import concourse.mybir as mybir

    pv_op, v_op = _register_custom_ops()
    nc = _single_act_set_bacc()("TRN2", target_bir_lowering=False, debug=False)
    f32 = mybir.dt.float32
    stock = nc.dram_tensor("stock_paths", [rows, time], f32, kind="ExternalInput").ap()
    vol = nc.dram_tensor("volume_paths", [rows, time], f32, kind="ExternalInput").ap()
    out = nc.dram_tensor("vwap_out", [rows, time], f32, kind="ExternalOutput").ap()

    Ln = mybir.ActivationFunctionType.Ln
    Exp = mybir.ActivationFunctionType.Exp

    width = width or time
    bpt = width // time  # DRAM row-blocks folded into one [P, width] tile
    n_tiles = rows // (P * bpt)

    def dma(dram_ap, sbuf_ap, store=False):
        if bpt == 1:
            args = (dram_ap, sbuf_ap[:]) if store else (sbuf_ap[:], dram_ap)
        else:
            d3 = dram_ap.rearrange("(b p) t -> p b t", b=bpt)
            s3 = sbuf_ap[:].rearrange("p (b t) -> p b t", b=bpt)
            args = (d3, s3) if store else (s3, d3)
        nc.sync.dma_start(*args)

    with tile.TileContext(nc) as tc:
        with (
            tc.tile_pool(name="big", bufs=bufs) as big,
            tc.tile_pool(name="small", bufs=bufs) as small,
        ):
            for i in range(n_tiles * reps):
                r0 = (i % n_tiles) * P * bpt
                rows_sl = slice(r0, r0 + P * bpt)
                ts = big.tile([P, width], f32, tag="ts")
                tv = big.tile([P, width], f32, tag="tv")
                dma(stock[rows_sl, :], ts)
                dma(vol[rows_sl, :], tv)
                t0 = small.tile([P, bpt], f32, tag="t0")
                for b in range(bpt):
                    nc.scalar.copy(t0[:, b : b + 1], ts[:, b * time : b * time + 1])
                for b in range(bpt):
                    sl = slice(b * time, (b + 1) * time)
                    nc.vector._custom_dve(pv_op, out=ts[:, sl], in0=ts[:, sl],
                                          in1=tv[:, sl])
                    nc.vector._custom_dve(v_op, out=tv[:, sl], in0=tv[:, sl])
                nc.scalar.activation(tv[:], tv[:], Ln)
                nc.scalar.activation(tv[:], tv[:], Exp, scale=-1.0)
                nc.vector.tensor_mul(ts[:], ts[:], tv[:])  # vwap
                for b in range(bpt):
                    nc.scalar.copy(ts[:, b * time : b * time + 1], t0[:, b : b + 1])
                dma(out[rows_sl, :], ts, store=True)
    nc.compile()
    return nc


def _get_nc():
    if "nc" not in _CACHE:
        _CACHE["nc"] = _build()
    return _CACHE["nc"]


def kernel(stock_paths: np.ndarray, volume_paths: np.ndarray) -> np.ndarray:
    from concourse.bass_utils import run_bass_kernel_spmd

    stock_paths = np.ascontiguousarray(stock_paths, dtype=np.float32)
    volume_paths = np.ascontiguousarray(volume_paths, dtype=np.float32)
    assert stock_paths.shape == (NUM_PATHS, TIME)

    nc = _get_nc()
    in_maps = [
        {
            "stock_paths": stock_paths[i * ROWS : (i + 1) * ROWS],
            "volume_paths": volume_paths[i * ROWS : (i + 1) * ROWS],
        }
        for i in range(N_CORES)
    ]
    res = run_bass_kernel_spmd(nc, in_maps, core_ids=list(range(N_CORES)))
    return np.concatenate([r["vwap_out"] for r in res.results], axis=0)
